# revision 2
# baseline (speedup 1.0000x reference)
"""Causal self-attention with RoPE on 8 Trainium2 NeuronCores — v2.

Data-parallel over batch (B=8 -> 1 per core). v2 is engine-rebalanced around
the Activation-engine exp bottleneck:

  - ACT runs ONLY exp (scale=1/8 fused). QK bias/downcast moves (PSUM->SBUF)
    go to DVE tensor_scalar.
  - Scores: the two heads of a pair are computed by row-tiled CONCURRENT
    matmuls (tile_position (0,0)/(64,0), K=64 each) into one psS tile
    [128, 1024] (h-major 512 halves = separate PSUM banks).
  - Causal mask applied by a PE matmul-accumulate of -200 onto the diagonal
    block (identity stationary), so exp needs no separate mask pass.
  - exp: ONE ACT instruction per k-tile covering both heads via a strided
    3D AP; writes pt bf16.
  - AV: stationary V_aug [128, 65] (ones column -> softmax denominator),
    accumulated per 512-q-window into psY [65, 512]; AV is software-pipelined
    one k-tile behind exp so the in-order PE queue never waits on ACT.
  - PSUM: pss 2 banks x2 bufs + psY 1 bank x 2 heads x 2 bufs = 8 exactly.
"""

import numpy as np
import ml_dtypes

import concourse.bass as bass
import concourse.tile as tile
from concourse import bacc, mybir
from concourse import bass_utils

F32 = mybir.dt.float32
BF16 = mybir.dt.bfloat16
FP8 = mybir.dt.float8e4
AF = mybir.ActivationFunctionType
ALU = mybir.AluOpType
PM = mybir.MatmulPerfMode

XSCALE = 16.0
WSCALE = 512.0
QKDESCALE = 1.0 / (XSCALE * WSCALE)

B, T, C = 8, 2048, 1024
H, HD = 16, 64
NCORES = 8

NT = T // 128      # 16 t-tiles
CCH = C // 128     # 8 c-chunks
NJT = 16           # 8 Q + 8 K row-tiles of the [2C, T] QK^T output
NHP = H // 2       # 8 head pairs


def _build(nc, repeat=1, parts=('p1', 'p2', 'p3'), loop_n=0):
    xt = nc.dram_tensor("xt", [C, T], BF16, kind="ExternalInput").ap()
    xt8 = nc.dram_tensor("xt8", [C, T], FP8, kind="ExternalInput").ap()
    wqk = nc.dram_tensor("wqk", [C, 2 * C], FP8, kind="ExternalInput").ap()
    wv = nc.dram_tensor("wv", [C, C], BF16, kind="ExternalInput").ap()
    wproj = nc.dram_tensor("wproj", [C, C], BF16, kind="ExternalInput").ap()
    cos4 = nc.dram_tensor("cos4", [128, T], BF16, kind="ExternalInput").ap()
    sin4s = nc.dram_tensor("sin4s", [128, T], BF16, kind="ExternalInput").ap()
    mad = nc.dram_tensor("mad", [128, 2, 128], BF16, kind="ExternalInput").ap()
    iden = nc.dram_tensor("iden", [128, 128], BF16, kind="ExternalInput").ap()
    bqk = nc.dram_tensor("bqk", [128, NJT], F32, kind="ExternalInput").ap()
    bvb = nc.dram_tensor("bvb", [128, C], BF16, kind="ExternalInput").ap()
    bpb = nc.dram_tensor("bpb", [128, C], F32, kind="ExternalInput").ap()
    y = nc.dram_tensor("y", [T, C], F32, kind="ExternalOutput").ap()
    rscr = nc.dram_tensor("rscr", [NHP * 8, 512], F32).ap()   # recip bounce

    from contextlib import ExitStack
    with tile.TileContext(nc) as tc, ExitStack() as ctx:
        ep = ctx.enter_context
        persist = ep(tc.tile_pool(name="persist", bufs=1))
        consts = ep(tc.tile_pool(name="consts", bufs=1))
        ph1 = ep(tc.tile_pool(name="ph1", bufs=1))
        wq_pool = ep(tc.tile_pool(name="wq_pool", bufs=3))
        qkp = ep(tc.tile_pool(name="qkp", bufs=2))
        rope_sw = ep(tc.tile_pool(name="rope_sw", bufs=2))
        rope_tmp = ep(tc.tile_pool(name="rope_tmp", bufs=1))
        pt_pool = ep(tc.tile_pool(name="pt_pool", bufs=5))
        den_pool = ep(tc.tile_pool(name="den", bufs=2))
        nrm_pool = ep(tc.tile_pool(name="nrm", bufs=2))
        yo_pool = ep(tc.tile_pool(name="yo_pool", bufs=2))
        wbig = ep(tc.tile_pool(name="wbig", bufs=1))
        ps_s = ep(tc.tile_pool(name="ps_s", bufs=2, space="PSUM"))
        ps_y = ep(tc.tile_pool(name="ps_y", bufs=2, space="PSUM"))

        cos_s = consts.tile([128, T], BF16)
        sin_s = consts.tile([128, T], BF16)
        mad_s = consts.tile([128, 2, 128], BF16)
        iden_s = consts.tile([128, 128], BF16)
        bqk_s = consts.tile([128, NJT], F32)
        bvb_s = consts.tile([128, C], BF16)
        bpb_s = consts.tile([128, C], F32)
        nc.sync.dma_start(out=cos_s, in_=cos4)
        nc.sync.dma_start(out=sin_s, in_=sin4s)
        nc.sync.dma_start(out=mad_s, in_=mad)
        nc.sync.dma_start(out=iden_s, in_=iden)
        nc.sync.dma_start(out=bqk_s, in_=bqk)
        nc.sync.dma_start(out=bvb_s, in_=bvb)
        nc.sync.dma_start(out=bpb_s, in_=bpb)

        def view3(t, qlo, n):
            """[128, 2, n] strided view of a [128, 1024] tile at col qlo."""
            return bass.AP(tensor=t.tensor, offset=t.offset + qlo,
                           ap=[[1024, 128], [512, 2], [1, n]])

        def qk_slot(hp, xts, which):
            """One 128-row tile of Q^T or K^T (2 heads) + RoPE. DVE moves."""
            jt = hp if which == 0 else 8 + hp
            dst = qkp.tile([128, T], BF16, tag="qk" + str(which),
                           name=f"qk{which}_{hp}")
            wt = wq_pool.tile([128, CCH, 128], FP8, tag="wqk", name=f"wt{jt}")
            nc.sync.dma_start(
                out=wt,
                in_=bass.AP(tensor=wqk.tensor, offset=wqk.offset + 128 * jt,
                            ap=[[2 * C, 128], [128 * 2 * C, CCH], [1, 128]]))
            for tck2 in range(2):
                ps = ps_s.tile([128, 1024], F32, tag="pss", name=f"psqk{tck2}")
                for half in range(2):
                    t0 = 1024 * tck2 + 512 * half
                    for ci2 in range(CCH // 2):
                        nc.tensor.matmul(
                            ps[:, 512 * half:512 * (half + 1)],
                            wt[:, 2 * ci2:2 * ci2 + 2, :],
                            xts[:, 2 * ci2:2 * ci2 + 2, t0:t0 + 512],
                            start=(ci2 == 0), stop=(ci2 == CCH // 2 - 1),
                            perf_mode=PM.DoubleRow)
                nc.scalar.activation(
                    dst[:, 1024 * tck2:1024 * (tck2 + 1)], ps,
                    AF.Identity, bias=bqk_s[:, jt:jt + 1], scale=QKDESCALE)
            sw = rope_sw.tile([128, T], BF16, tag="sw", name=f"sw{jt}")
            nc.sync.dma_start(out=sw[0:32, :], in_=dst[32:64, :])
            nc.sync.dma_start(out=sw[32:64, :], in_=dst[0:32, :])
            nc.sync.dma_start(out=sw[64:96, :], in_=dst[96:128, :])
            nc.sync.dma_start(out=sw[96:128, :], in_=dst[64:96, :])
            tmp = rope_tmp.tile([128, T], BF16, tag="tmp", name=f"tmp{jt}")
            nc.vector.tensor_mul(tmp, dst, cos_s)
            nc.vector.tensor_mul(sw, sw, sin_s)
            nc.vector.tensor_add(dst, tmp, sw)
            return dst

        def attention(hp, qtile, ktile, vs, yc):
            for qw in range(4):          # 512-wide q-windows
                q_base = 512 * qw
                kmax = 4 * (qw + 1)
                psY = [ps_y.tile([65, 512], F32, tag=f"psy{h}",
                                 name=f"psY{h}_{qw % 2}") for h in range(2)]

                def av(prev, last):
                    pkt, ppt, pqlo = prev
                    for h in range(2):
                        nc.tensor.matmul(
                            psY[h][:, pqlo:512],
                            vs[pkt][:, 2 * hp + h, :],
                            ppt[:, 512 * h + pqlo:512 * h + 512],
                            start=(pkt == 0), stop=last,
                            skip_group_check=True)

                prev = None
                for kt in range(kmax):
                    k0 = 128 * kt
                    qlo = max(0, k0 - q_base)
                    psS = ps_s.tile([128, 1024], F32, tag="pss",
                                    name=f"psS{kt % 2}")
                    nc.tensor.matmul(
                        psS[:, qlo:512],
                        ktile[0:64, k0:k0 + 128],
                        qtile[0:64, q_base + qlo:q_base + 512],
                        start=True, stop=True, tile_position=(0, 0),
                        skip_group_check=True)
                    nc.tensor.matmul(
                        psS[:, 512 + qlo:1024],
                        ktile[64:128, k0:k0 + 128],
                        qtile[64:128, q_base + qlo:q_base + 512],
                        start=True, stop=True, tile_position=(64, 0),
                        skip_group_check=True)
                    if k0 >= q_base:
                        nc.tensor.matmul(
                            view3(psS, qlo, 128), iden_s, mad_s,
                            start=False, stop=True, skip_group_check=True)
                    pt = pt_pool.tile([128, 1024], BF16, tag="pt",
                                      name=f"pt{kt % 5}")
                    if qlo == 0:
                        nc.scalar.activation(pt, psS, AF.Exp, scale=0.125)
                    else:
                        nc.scalar.activation(
                            view3(pt, qlo, 512 - qlo),
                            view3(psS, qlo, 512 - qlo), AF.Exp, scale=0.125)
                    if prev is not None:
                        av(prev, last=False)
                    prev = (kt, pt, qlo)
                av(prev, last=True)

                for h in range(2):
                    i = hp * 8 + qw * 2 + h
                    rec = den_pool.tile([1, 512], F32, tag="rec",
                                        name=f"rec{i % 2}")
                    nc.vector.reciprocal(rec, psY[h][64:65, :])
                    nc.sync.dma_start(out=rscr[i:i + 1, :], in_=rec)
                    rb = nrm_pool.tile([64, 512], F32, tag="rb",
                                       name=f"rb{i % 2}")
                    nc.sync.dma_start(
                        out=rb, in_=rscr[i:i + 1, :].partition_broadcast(64))
                    nc.vector.tensor_mul(
                        yc[hp][64 * h:64 * h + 64, q_base:q_base + 512],
                        psY[h][0:64, :], rb)

        def body():
            vs = [persist.tile([128, H, HD + 1], BF16, tag=f"vs{v}",
                               name=f"vs{v}") for v in range(NT)]
            yc = [persist.tile([128, T], BF16, tag=f"yc{s}", name=f"yc{s}")
                  for s in range(NHP)]
            xts = ph1.tile([128, CCH, T], BF16, tag="xts", name="xts")
            xts8 = ph1.tile([128, CCH, T], FP8, tag="xts8", name="xts8")
            if 'p1' in parts:
                for ci in range(CCH):
                    nc.sync.dma_start(
                        out=xts[:, ci, :], in_=xt[128 * ci:128 * (ci + 1), :])
                    nc.sync.dma_start(
                        out=xts8[:, ci, :], in_=xt8[128 * ci:128 * (ci + 1), :])
                for v in range(NT):
                    nc.vector.memset(vs[v][:, :, HD:HD + 1], 1.0)
                wvt = wbig.tile([128, CCH, 2, 512], BF16, tag="wbig",
                                name="wvt")
                nc.sync.dma_start(
                    out=wvt,
                    in_=bass.AP(tensor=wv.tensor, offset=wv.offset,
                                ap=[[C, 128], [128 * C, CCH], [512, 2], [1, 512]]))
                for vt in range(NT):
                    ps = ps_s.tile([128, 1024], F32, tag="pss", name=f"psv{vt}")
                    for nck in range(2):
                        for ci in range(CCH):
                            nc.tensor.matmul(
                                ps[:, 512 * nck:512 * (nck + 1)],
                                xts[:, ci, 128 * vt:128 * (vt + 1)],
                                wvt[:, ci, nck, :],
                                start=(ci == 0), stop=(ci == CCH - 1))
                    nc.vector.tensor_add(vs[vt][:, :, 0:HD], ps, bvb_s)

            for hp in range(NHP):
                if 'p1' in parts:
                    qtile = qk_slot(hp, xts8, 0)
                    ktile = qk_slot(hp, xts8, 1)
                else:
                    qtile = qkp.tile([128, T], BF16, tag="qk0", name="qk0d")
                    ktile = qkp.tile([128, T], BF16, tag="qk1", name="qk1d")
                    nc.vector.memset(qtile[:, 0:8], 0.0)
                    nc.vector.memset(ktile[:, 0:8], 0.0)
                if 'p2' in parts:
                    attention(hp, qtile, ktile, vs, yc)

            if 'p2' not in parts:
                for s in range(NHP):
                    nc.vector.memset(yc[s][:, 0:8], 0.0)
            if 'p1' not in parts:
                for v in range(NT):
                    nc.vector.memset(vs[v][:, 0, 0:8], 0.0)

            # ---------------- output projection ------------------------
            if 'p3' in parts:
                wpt = wbig.tile([128, CCH, 2, 512], BF16, tag="wbig",
                                name="wpt")
                nc.sync.dma_start(
                    out=wpt,
                    in_=bass.AP(tensor=wproj.tensor, offset=wproj.offset,
                                ap=[[C, 128], [128 * C, CCH], [512, 2], [1, 512]]))
                for tt in range(NT):
                    ps = ps_s.tile([128, 1024], F32, tag="pss", name=f"psp{tt}")
                    for ec in range(2):
                        for ci in range(CCH):
                            nc.tensor.matmul(
                                ps[:, 512 * ec:512 * (ec + 1)],
                                yc[ci][:, 128 * tt:128 * (tt + 1)],
                                wpt[:, ci, ec, :],
                                start=(ci == 0), stop=(ci == CCH - 1))
                    yo = yo_pool.tile([128, 1024], F32, tag="yo",
                                      name=f"yo{tt}")
                    nc.vector.tensor_add(yo, ps, bpb_s)
                    nc.sync.dma_start(
                        out=y[128 * tt:128 * (tt + 1), :], in_=yo)

        if loop_n > 0:
            with tc.For_i(0, loop_n, 1) as _i:
                for _rep in range(repeat):
                    body()
        else:
            for _rep in range(repeat):
                body()
    return nc


_PERM = None


def _head_perm():
    global _PERM
    if _PERM is None:
        within = np.concatenate([np.arange(0, HD, 2), np.arange(1, HD, 2)])
        _PERM = (np.arange(H)[:, None] * HD + within[None, :]).reshape(-1)
    return _PERM


def _prep_shared(freqs, W_attn, b_attn, W_proj, b_proj):
    bf = ml_dtypes.bfloat16
    f8 = ml_dtypes.float8_e4m3
    perm = _head_perm()
    wq = W_attn[:, 0:C][:, perm]
    wk = W_attn[:, C:2 * C][:, perm]
    wqk = np.clip(np.concatenate([wq, wk], axis=1) * WSCALE,
                  -240, 240).astype(f8)
    wv = np.ascontiguousarray(W_attn[:, 2 * C:3 * C]).astype(bf)
    wproj = np.ascontiguousarray(W_proj).astype(bf)

    cos = np.cos(freqs.astype(np.float64)).astype(np.float32)   # [T, 32]
    sin = np.sin(freqs.astype(np.float64)).astype(np.float32)
    cos4 = np.empty((128, T), np.float32)
    sin4s = np.empty((128, T), np.float32)
    for blk in range(4):
        cos4[32 * blk:32 * blk + 32] = cos.T
        sgn = -1.0 if blk % 2 == 0 else 1.0
        sin4s[32 * blk:32 * blk + 32] = sgn * sin.T

    # mask-add: -200 on strictly-lower triangle (k > q), duplicated per head
    m1 = np.where(np.arange(128)[:, None] <= np.arange(128)[None, :],
                  0.0, -200.0).astype(np.float32)
    mad = np.stack([m1, m1], axis=1)            # [128, 2, 128]
    iden = np.eye(128, dtype=np.float32)

    bq = b_attn[0:C][perm]
    bk = b_attn[C:2 * C][perm]
    bqk = np.concatenate([bq, bk]).reshape(NJT, 128).T.astype(np.float32)
    bqk = np.ascontiguousarray(bqk)
    bvb = np.broadcast_to(b_attn[2 * C:3 * C], (128, C)).astype(np.float32)
    bpb = np.broadcast_to(b_proj, (128, C)).astype(np.float32)
    return {
        "wqk": wqk, "wv": wv, "wproj": wproj,
        "cos4": cos4.astype(bf), "sin4s": sin4s.astype(bf),
        "mad": np.ascontiguousarray(mad).astype(bf),
        "iden": np.ascontiguousarray(iden).astype(bf),
        "bqk": bqk,
        "bvb": np.ascontiguousarray(bvb).astype(bf),
        "bpb": np.ascontiguousarray(bpb),
    }


_CACHE = {}


def _get_nc():
    if "nc" not in _CACHE:
        nc = bacc.Bacc("TRN2", target_bir_lowering=False, debug=False,
                       num_devices=NCORES)
        _build(nc)
        nc.compile()
        _CACHE["nc"] = nc
    return _CACHE["nc"]


def kernel(x, freqs, W_attn, b_attn, W_proj, b_proj, **_unused):
    x = np.asarray(x, dtype=np.float32)
    shared = _prep_shared(
        np.asarray(freqs, np.float32), np.asarray(W_attn, np.float32),
        np.asarray(b_attn, np.float32), np.asarray(W_proj, np.float32),
        np.asarray(b_proj, np.float32))
    bf = ml_dtypes.bfloat16
    f8 = ml_dtypes.float8_e4m3
    in_maps = []
    for b in range(NCORES):
        xT = np.ascontiguousarray(x[b].T)                # [C, T]
        xtb = xT.astype(bf)
        xtb8 = np.clip(xT * XSCALE, -240, 240).astype(f8)
        in_maps.append({"xt": xtb, "xt8": xtb8, **shared})

    nc = _get_nc()
    res = bass_utils.run_bass_kernel_spmd(nc, in_maps, core_ids=list(range(NCORES)))
    out = np.stack([res.results[b]["y"] for b in range(NCORES)], axis=0)
    return out.astype(np.float32)
